# revision 3
# baseline (speedup 1.0000x reference)
"""Node2Node supervised-contrastive loss on 8 Trainium2 NeuronCores — v2.

Data-parallel over the sample table (same routing as the baseline), with:
  - slim feature table [SLP, EL] bf16 (EL=128 by default: 256B gather rows)
  - positive/valid masks precomputed host-side (integer label compares),
    so no label gather / one-hot machinery on device
  - positives-count per anchor supplied to k2 from host bookkeeping
  - sidx/masks prefetched to SBUF once, so phase C issues only gathers
  - k1 outputs [128, NB, 2] (num, den) partial sums per anchor slot
"""

import os
import sys

import numpy as np
import ml_dtypes

sys.path.insert(0, "/opt/trn_rl_repo")

import concourse.bass as bass
import concourse.bacc as bacc
import concourse.mybir as mybir
import concourse.tile as tile
from concourse import bass_utils

F32 = mybir.dt.float32
BF16 = mybir.dt.bfloat16
I16 = mybir.dt.int16
I32 = mybir.dt.int32
MUL = mybir.AluOpType.mult
ADD = mybir.AluOpType.add
SUB = mybir.AluOpType.subtract
EQ = mybir.AluOpType.is_equal
AFT = mybir.ActivationFunctionType


class CFG:
    def __init__(self, N=100000, D=128, A=4096, S=512, NC=8, TEMP=0.1,
                 MT=104, EL=128, NQ=4, SPL=4, BUFS=2):
        self.N, self.D, self.A, self.S, self.NC, self.TEMP = N, D, A, S, NC, TEMP
        self.SL = N // NC                      # rows per slice
        self.NB = A // 128                     # anchor blocks (slots of 128)
        self.G = -(-self.SL // 128)            # slice col-groups of 128 rows
        self.SLP = self.G * 128                # padded slice rows
        self.MT = MT                           # max columns per gather call
        self.EL = EL                           # table row elems (>=D, 128-mult)
        self.NQ = NQ                           # swdge queues for gathers
        self.SPL = SPL                         # sub-gathers per tile
        self.BUFS = BUFS                       # gather tile double-buffering


REAL = CFG(MT=int(os.environ.get("K2_MT", "80")),
           EL=int(os.environ.get("K2_EL", "128")),
           NQ=int(os.environ.get("K2_NQ", "4")),
           SPL=int(os.environ.get("K2_SPL", "2")),
           BUFS=int(os.environ.get("K2_BUFS", "5")))


# --------------------------------------------------------------------------
# host-side index prep (pure numpy; integer bookkeeping only)
# --------------------------------------------------------------------------

def prep(cfg, x, y, anchors, sampled):
    N, A, S, NC, SL, NB = cfg.N, cfg.A, cfg.S, cfg.NC, cfg.SL, cfg.NB
    x = np.ascontiguousarray(np.asarray(x, dtype=np.float32))
    y64 = np.asarray(y, dtype=np.int64)
    anchors = np.asarray(anchors, dtype=np.int64)
    sampled = np.asarray(sampled, dtype=np.int64)

    ya = y64[anchors]                          # [A]
    ys = y64[sampled]                          # [A, S]
    pos = ys == ya[:, None]                    # [A, S] bool
    cnt_pos = pos.sum(1).astype(np.float32)    # [A] positives per anchor

    core_of = sampled // SL                    # [A, S]
    cnt = np.zeros((A, NC), dtype=np.int64)
    for c in range(NC):
        cnt[:, c] = (core_of == c).sum(1)

    perms, ranks = [], []
    Ms = np.zeros((NC, NB), dtype=np.int64)
    for c in range(NC):
        p = np.argsort(cnt[:, c], kind="stable")
        r = np.empty(A, dtype=np.int64)
        r[p] = np.arange(A)
        perms.append(p)
        ranks.append(r)
        Ms[c] = cnt[p, c].reshape(NB, 128).max(1)
    M = Ms.max(0)                              # uniform per-block columns
    Cj = np.concatenate([[0], np.cumsum(M)])
    MTOT = int(Cj[-1])

    cores = []
    for c in range(NC):
        perm, rank = perms[c], ranks[c]
        a_list, s_list = np.nonzero(core_of == c)       # sorted by anchor
        local = (sampled[a_list, s_list] - c * SL).astype(np.int64)
        n = cnt[:, c]
        start = np.concatenate([[0], np.cumsum(n)])
        k = np.arange(len(a_list)) - start[a_list]      # within-anchor position
        r = rank[a_list]
        j, p = r // 128, r % 128
        col = Cj[j] + k
        idxmat = np.zeros((128, MTOT), dtype=np.int16)
        vmask = np.zeros((128, MTOT), dtype=ml_dtypes.bfloat16)
        pmask = np.zeros((128, MTOT), dtype=ml_dtypes.bfloat16)
        idxmat[p, col] = local.astype(np.int16)
        vmask[p, col] = 1.0
        pmask[p, col] = pos[a_list, s_list].astype(ml_dtypes.bfloat16)
        flat = idxmat.T.reshape(-1)                     # [MTOT*128]
        L = flat.size // 16
        wrapped = np.zeros((128, L), dtype=np.int16)
        w16 = flat.reshape(L, 16).T
        for g in range(8):
            wrapped[g * 16:(g + 1) * 16, :] = w16

        aperm = anchors[perm]                           # [A] node ids, slot order
        xa = x[aperm].reshape(NB, 128, cfg.D).transpose(1, 0, 2).copy()

        xs = np.ones((cfg.SLP, cfg.D), dtype=np.float32)
        xs[:SL] = x[c * SL:(c + 1) * SL]

        cores.append(dict(xs=xs, xa=xa, sidx=wrapped, vmask=vmask,
                          pmask=pmask))
    return cores, perms, M.astype(int).tolist(), cnt_pos


# --------------------------------------------------------------------------
# kernel 1: per-core partial sums
# --------------------------------------------------------------------------

def build_k1(cfg, M, repeat=1):
    NB, D, G, SLP, MT, EL = cfg.NB, cfg.D, cfg.G, cfg.SLP, cfg.MT, cfg.EL
    MTOT = sum(M)
    nc = bacc.Bacc("TRN2", target_bir_lowering=False, debug=False,
                   num_devices=cfg.NC, num_swdge_queues=cfg.NQ)
    xs = nc.dram_tensor("xs", [SLP, D], F32, kind="ExternalInput").ap()
    xa = nc.dram_tensor("xa", [128, NB, D], F32, kind="ExternalInput").ap()
    sidx = nc.dram_tensor("sidx", [128, MTOT * 8], I16, kind="ExternalInput").ap()
    vmask = nc.dram_tensor("vmask", [128, MTOT], BF16, kind="ExternalInput").ap()
    pmask = nc.dram_tensor("pmask", [128, MTOT], BF16, kind="ExternalInput").ap()
    acc_out = nc.dram_tensor("acc", [128, NB, 2], F32, kind="ExternalOutput").ap()

    with tile.TileContext(nc) as tc:
        with tc.tile_pool(name="dram", bufs=1, space="DRAM") as dpool, \
             tc.tile_pool(name="pre", bufs=1) as pre, \
             tc.tile_pool(name="res", bufs=1) as res:
            table = dpool.tile([SLP, EL], BF16)

            # prefetch all phase-C indices and masks up front
            it_all = pre.tile([128, MTOT * 8], I16)
            nc.sync.dma_start(it_all[:], sidx[:])
            vm_all = pre.tile([128, MTOT], BF16)
            nc.sync.dma_start(vm_all[:], vmask[:])
            pm_all = pre.tile([128, MTOT], BF16)
            nc.sync.dma_start(pm_all[:], pmask[:])

            # ---- phase A: build normalized bf16 slice table ----
            with tc.tile_pool(name="pa", bufs=2) as pa:
                half = (G + 1) // 2
                xsr = xs.rearrange("(g p) d -> p g d", p=128)
                tbr = table[:].rearrange("(g p) e -> p g e", p=128)
                for h in range(2):
                    g0 = h * half
                    g1 = min(G, g0 + half)
                    gw = g1 - g0
                    if gw <= 0:
                        continue
                    xt = pa.tile([128, half, D], F32, tag="xt")
                    nc.sync.dma_start(xt[:, :gw, :], xsr[:, g0:g1, :])
                    sq = pa.tile([128, half, D], F32, tag="sq")
                    nc.vector.tensor_tensor(out=sq[:, :gw, :], in0=xt[:, :gw, :],
                                            in1=xt[:, :gw, :], op=MUL)
                    ss = pa.tile([128, half], F32, tag="ss")
                    nc.vector.reduce_sum(out=ss[:, :gw], in_=sq[:, :gw, :],
                                         axis=mybir.AxisListType.X)
                    nc.scalar.activation(ss[:, :gw], ss[:, :gw], AFT.Sqrt)
                    inv = pa.tile([128, half], F32, tag="inv")
                    nc.vector.reciprocal(inv[:, :gw], ss[:, :gw])
                    tb = pa.tile([128, half, EL], BF16, tag="tb")
                    if EL > D:
                        nc.vector.memset(tb[:, :gw, D:], 0.0)
                    nc.vector.tensor_tensor(
                        out=tb[:, :gw, 0:D], in0=xt[:, :gw, :],
                        in1=inv[:, :gw].unsqueeze(2).to_broadcast([128, gw, D]),
                        op=MUL)
                    nc.sync.dma_start(tbr[:, g0:g1, :], tb[:, :gw, :])

            # ---- phase B: anchor features (slot layout) ----
            with tc.tile_pool(name="pb", bufs=1) as pb:
                xat = pb.tile([128, NB, D], F32)
                nc.sync.dma_start(xat[:], xa[:])
                sqa = pb.tile([128, NB, D], F32)
                nc.vector.tensor_tensor(out=sqa[:], in0=xat[:], in1=xat[:], op=MUL)
                ssa = pb.tile([128, NB], F32)
                nc.vector.reduce_sum(out=ssa[:], in_=sqa[:],
                                     axis=mybir.AxisListType.X)
                nc.scalar.activation(ssa[:], ssa[:], AFT.Sqrt)
                inva = pb.tile([128, NB], F32)
                nc.vector.reciprocal(inva[:], ssa[:])
                af = res.tile([128, NB, D], BF16)
                nc.vector.tensor_tensor(
                    out=af[:], in0=xat[:],
                    in1=inva[:].unsqueeze(2).to_broadcast([128, NB, D]), op=MUL)

            acc = res.tile([128, NB, 2], F32)
            nc.vector.memset(acc[:], 0.0)

            # ---- phase C: main pair loop ----
            # gather tiles are fixed-width and span anchor-block boundaries;
            # only the anchor multiply and the per-block reductions split.
            Cj = np.concatenate([[0], np.cumsum(M)]).astype(int)

            def overlaps(c0, mt):
                out = []
                for j in range(NB):
                    a, b = max(c0, Cj[j]), min(c0 + mt, Cj[j + 1])
                    if b > a:
                        out.append((j, a - c0, b - c0))
                return out

            with tc.tile_pool(name="pcb", bufs=cfg.BUFS) as pcb, \
                 tc.tile_pool(name="pcp", bufs=2) as pcp, \
                 tc.tile_pool(name="pc", bufs=3) as pc:
                gather_i = 0
                for _rep in range(repeat):
                    c0 = 0
                    while c0 < MTOT:
                        mt = min(MT, MTOT - c0)
                        st = pcb.tile([128, MT, EL], BF16, tag="st")
                        # split into SPL sub-gathers on distinct queues so
                        # several SDMA rings run concurrently per tile
                        edges = [round(s * mt / cfg.SPL)
                                 for s in range(cfg.SPL + 1)]
                        for s in range(cfg.SPL):
                            c1, c2 = edges[s], edges[s + 1]
                            if c2 == c1:
                                continue
                            nc.gpsimd.dma_gather(
                                st[:, c1:c2, :], table[:],
                                it_all[:, (c0 + c1) * 8:(c0 + c2) * 8],
                                (c2 - c1) * 128, (c2 - c1) * 128, EL,
                                single_packet=False,
                                queue_num=gather_i % cfg.NQ)
                            gather_i += 1
                        # product into a separate scratch pool so st is
                        # released right after the multiply (keeps the
                        # gather pipeline deep); tree-reduce in place in pr
                        pr = pcp.tile([128, MT, D], BF16, tag="pr")
                        for j, a, b in overlaps(c0, mt):
                            nc.vector.tensor_tensor(
                                out=pr[:, a:b, :], in0=st[:, a:b, 0:D],
                                in1=af[:, j:j + 1, :].to_broadcast(
                                    [128, b - a, D]),
                                op=MUL)
                        w = D // 2
                        while w >= 1:
                            nc.vector.tensor_tensor(
                                out=pr[:, :mt, 0:w], in0=pr[:, :mt, 0:w],
                                in1=pr[:, :mt, w:2 * w], op=ADD)
                            w //= 2
                        e = pc.tile([128, MT], F32, tag="e")
                        nc.scalar.activation(e[:, :mt], pr[:, :mt, 0],
                                             AFT.Exp, scale=1.0 / cfg.TEMP)
                        em = pc.tile([128, MT], F32, tag="em")
                        nc.vector.tensor_tensor(
                            out=em[:, :mt], in0=e[:, :mt],
                            in1=pm_all[:, c0:c0 + mt], op=MUL)
                        ev = pc.tile([128, MT], F32, tag="ev")
                        nc.vector.tensor_tensor(
                            out=ev[:, :mt], in0=e[:, :mt],
                            in1=vm_all[:, c0:c0 + mt], op=MUL)
                        for q, src in ((0, em), (1, ev)):
                            for j, a, b in overlaps(c0, mt):
                                tmp = pc.tile([128, 1], F32, tag=f"tmp{q}")
                                nc.vector.reduce_sum(
                                    out=tmp[:], in_=src[:, a:b],
                                    axis=mybir.AxisListType.X)
                                nc.vector.tensor_tensor(
                                    out=acc[:, j, q:q + 1],
                                    in0=acc[:, j, q:q + 1],
                                    in1=tmp[:], op=ADD)
                        c0 += mt
            nc.sync.dma_start(acc_out[:], acc[:])
    nc.compile()
    return nc


# --------------------------------------------------------------------------
# kernel 2: combine partials, per-anchor loss, total
# --------------------------------------------------------------------------

def build_k2(cfg):
    NB, NC = cfg.NB, cfg.NC
    nc = bacc.Bacc("TRN2", target_bir_lowering=False, debug=False, num_devices=1)
    parts = nc.dram_tensor("parts", [128, NC, NB, 2], F32,
                           kind="ExternalInput").ap()
    cntin = nc.dram_tensor("cnt", [128, NB], F32, kind="ExternalInput").ap()
    out = nc.dram_tensor("out", [1, 1], F32, kind="ExternalOutput").ap()
    with tile.TileContext(nc) as tc:
        with tc.tile_pool(name="p", bufs=1) as p, \
             tc.tile_pool(name="ps", bufs=1, space="PSUM") as psp:
            t = p.tile([128, NC, NB, 2], F32)
            nc.sync.dma_start(t[:], parts[:])
            c_ = p.tile([128, NB], F32)
            nc.sync.dma_start(c_[:], cntin[:])
            s2 = p.tile([128, NB, 2], F32)
            tt = t[:].transpose([0, 2, 3, 1])
            nc.vector.reduce_sum(out=s2[:], in_=tt, axis=mybir.AxisListType.X)
            n_ = s2[:, :, 0]
            d_ = s2[:, :, 1]
            cz = p.tile([128, NB], F32)
            nc.vector.tensor_scalar(out=cz[:], in0=c_[:], scalar1=0.0,
                                    scalar2=None, op0=EQ)
            n1 = p.tile([128, NB], F32)
            nc.vector.tensor_tensor(out=n1[:], in0=n_, in1=cz[:], op=ADD)
            c1 = p.tile([128, NB], F32)
            nc.vector.tensor_scalar_max(out=c1[:], in0=c_[:], scalar1=1.0)
            lnn = p.tile([128, NB], F32)
            nc.scalar.activation(lnn[:], n1[:], AFT.Ln)
            lnd = p.tile([128, NB], F32)
            nc.scalar.activation(lnd[:], d_, AFT.Ln)
            df = p.tile([128, NB], F32)
            nc.vector.tensor_tensor(out=df[:], in0=lnd[:], in1=lnn[:], op=SUB)
            rc = p.tile([128, NB], F32)
            nc.vector.reciprocal(rc[:], c1[:])
            pa = p.tile([128, NB], F32)
            nc.vector.tensor_tensor(out=pa[:], in0=df[:], in1=rc[:], op=MUL)
            m = p.tile([128, NB], F32)
            nc.scalar.activation(m[:], cz[:], AFT.Copy, scale=-1.0, bias=1.0)
            pa2 = p.tile([128, NB], F32)
            nc.vector.tensor_tensor(out=pa2[:], in0=pa[:], in1=m[:], op=MUL)
            rs = p.tile([128, 1], F32)
            nc.vector.reduce_sum(out=rs[:], in_=pa2[:], axis=mybir.AxisListType.X)
            ones = p.tile([128, 1], F32)
            nc.vector.memset(ones[:], 1.0)
            accp = psp.tile([1, 1], F32)
            nc.tensor.matmul(out=accp[:], lhsT=rs[:], rhs=ones[:], start=True,
                             stop=True)
            res_t = p.tile([1, 1], F32)
            nc.vector.tensor_copy(out=res_t[:], in_=accp[:])
            nc.sync.dma_start(out[:], res_t[:])
    nc.compile()
    return nc


# --------------------------------------------------------------------------
# entry point
# --------------------------------------------------------------------------

def _run(cfg, x, y, anchors, sampled, repeat=1):
    cores, perms, M, cnt_pos = prep(cfg, x, y, anchors, sampled)
    nc1 = build_k1(cfg, M, repeat=repeat)
    in_maps = [dict(xs=c["xs"], xa=c["xa"], sidx=c["sidx"], vmask=c["vmask"],
                    pmask=c["pmask"]) for c in cores]
    r1 = bass_utils.run_bass_kernel_spmd(nc1, in_maps,
                                         core_ids=list(range(cfg.NC)))
    aligned = np.zeros((cfg.NC, cfg.A, 2), dtype=np.float32)
    for c in range(cfg.NC):
        acc = r1.results[c]["acc"]                        # [128, NB, 2]
        acc_t = acc.transpose(1, 0, 2).reshape(cfg.A, 2)  # slot-rank order
        aligned[c, perms[c]] = acc_t
    parts = aligned.reshape(cfg.NC, cfg.NB, 128, 2).transpose(2, 0, 1, 3).copy()
    cnt_plane = (cnt_pos.reshape(cfg.NB, 128).T.copy() * repeat).astype(
        np.float32)
    nc2 = build_k2(cfg)
    r2 = bass_utils.run_bass_kernel_spmd(
        nc2, [dict(parts=parts, cnt=cnt_plane)], core_ids=[0])
    val = np.float32(r2.results[0]["out"].reshape(-1)[0]) * repeat
    return val, r1, aligned


def kernel(x, y, anchors, sampled):
    val, _, _ = _run(REAL, np.asarray(x), np.asarray(y), np.asarray(anchors),
                     np.asarray(sampled))
    return np.asarray(val, dtype=np.float32)
